# revision 8
# baseline (speedup 1.0000x reference)
"""Trainium2 Bass kernel for a 4-layer transformer encoder (v3, bf16).

Shapes: B=2, T=2048, D=1024, H=16, DH=64, DFF=4096, L=4.

Sharding: token-parallel over 8 cores (512 tokens/core; batch b on cores
4b..4b+3). Per layer: K,V projections (bf16) -> HBM bounce -> AllGather
within the 4-core group; Q local; full attention for the core's 512
queries; Wo/LN/FFN local.

v3 changes vs v2:
 - V is stored/shipped in an attention-friendly interleaved layout
   [P, pair(8), t(4), head(2), 65] with the softmax-denominator ones
   column baked in, so every attention-side V load is one contiguous
   [P, 520] DMA (v2 shredded ~350k tiny descriptors here).
 - 2 AllGathers per layer (K full, V full) instead of 4 half-gathers.
 - The softmax-denominator reciprocal runs on a [32, 32] reshape of the
   sums row (DMA reshape -> DVE) instead of a [1, 1024] single-partition
   DVE reciprocal (6.5 us -> ~0.3 us each).
"""

import sys

sys.path.insert(0, "/opt/trn_rl_repo")

import numpy as np
import ml_dtypes

import concourse.bacc as bacc
import concourse.mybir as mybir
import concourse.tile as tile
from concourse.bass_utils import run_bass_kernel_spmd

F32 = mybir.dt.float32
BF16 = mybir.dt.bfloat16
F32R = mybir.dt.float32r
AF = mybir.ActivationFunctionType
ALU = mybir.AluOpType
BF16NP = ml_dtypes.bfloat16

L, D, H, DH, DFF = 4, 1024, 16, 64, 4096
B, T = 2, 2048
EPS = 1e-5
NCORES = 8
GROUP = 4
TOK = (B * T) // NCORES   # 512
P = 128
KD = D // P               # 8
KF = DFF // P             # 32
NKT = T // P              # 16 global key tiles
NPAIR = H // 2            # 8
VW = 520                  # per-pair V block: 4 t * (2 heads * 65)
VSB = NPAIR * VW          # 4160

# vecs column layout (per layer), all fp32
C_BQ, C_BK, C_BATT, C_B1, C_B2 = 0, 8, 16, 24, 56
C_LG1, C_LB1, C_LG2, C_LB2 = 64, 72, 80, 88
NVEC = 96

_PROGRAM = None
LAST_RES = None


def r32(ap):
    return ap.bitcast(F32R)


def _build_program():
    nc = bacc.Bacc("TRN2", target_bir_lowering=False, debug=False,
                   num_devices=NCORES)

    xT = nc.dram_tensor("xT", [KD, P, TOK], F32, kind="ExternalInput").ap()
    x16 = nc.dram_tensor("x16", [KD, P, TOK], BF16, kind="ExternalInput").ap()
    mb = nc.dram_tensor("mb", [P, NKT], F32, kind="ExternalInput").ap()
    wq = nc.dram_tensor("wq", [L, KD, P, D], BF16, kind="ExternalInput").ap()
    wk = nc.dram_tensor("wk", [L, KD, P, D], BF16, kind="ExternalInput").ap()
    wv = nc.dram_tensor("wv", [L, KD, P, D], BF16, kind="ExternalInput").ap()
    wo = nc.dram_tensor("wo", [L, KD, P, D], BF16, kind="ExternalInput").ap()
    w1 = nc.dram_tensor("w1", [L, KF, P, D], BF16, kind="ExternalInput").ap()
    w2 = nc.dram_tensor("w2", [L, KD, 4, P, 1024], BF16,
                        kind="ExternalInput").ap()
    vecs = nc.dram_tensor("vecs", [L, P, NVEC], F32, kind="ExternalInput").ap()
    vecf = nc.dram_tensor("vecf", [P, 16], F32, kind="ExternalInput").ap()
    onesd = nc.dram_tensor("onesd", [P, 8], F32, kind="ExternalInput").ap()
    ones16d = nc.dram_tensor("ones16d", [P, 8], BF16,
                             kind="ExternalInput").ap()
    outT = nc.dram_tensor("outT", [KD, P, TOK], F32, kind="ExternalOutput").ap()

    rg = [[0, 1, 2, 3], [4, 5, 6, 7]]

    with tile.TileContext(nc) as tc:
        with tc.tile_pool(name="sb", bufs=1) as sb, \
             tc.tile_pool(name="ps", bufs=1, space="PSUM") as psp, \
             tc.tile_pool(name="dr", bufs=1, space="DRAM") as dr:

            ones_col = sb.tile([P, 1], F32, tag="ones_col", bufs=1)
            nc.sync.dma_start(r32(ones_col[:]), r32(onesd[:, 0:1]))
            ones_row = sb.tile([1, P], F32, tag="ones_row", bufs=1)
            nc.sync.dma_start(r32(ones_row[:]),
                              r32(onesd[:, 0:1].rearrange("p o -> o p")))
            mb_t = sb.tile([P, NKT], F32, tag="mb", bufs=1)
            nc.sync.dma_start(mb_t[:], mb[:])

            # residual-base (x-tilde) fp32 tiles + bf16 matmul view
            xt = []    # fp32 [P, TOK] x 8
            xh = []    # bf16 [P, TOK] x 8
            for k in range(KD):
                t32 = sb.tile([P, TOK], F32, tag="res", bufs=16,
                              name=f"x0_{k}")
                nc.sync.dma_start(r32(t32[:]), r32(xT[k]))
                xt.append(t32)
                t16 = sb.tile([P, TOK], BF16, tag=f"xh{k}", bufs=1,
                              name=f"xh0_{k}")
                nc.sync.dma_start(t16[:], x16[k])
                xh.append(t16)

            vec_t = None
            dumt = sb.tile([1, 8], F32, tag="dumt", bufs=1)
            nc.vector.tensor_copy(dumt[:], ones_row[:, 0:8])

            def proj(l, w_ap, bias_col, dst, tag):
                """dst[:, o*TOK:(o+1)*TOK] = (W @ x)^T + bias, bf16."""
                for o in range(KD):
                    wt = sb.tile([P, D], BF16, tag="wproj", bufs=8,
                                 name=f"w_{tag}_{l}_{o}")
                    nc.sync.dma_start(wt[:], w_ap[l, o])
                    ps = psp.tile([P, TOK], F32, tag="psA", bufs=2,
                                  name=f"ps_{tag}_{l}_{o}")
                    for k in range(KD):
                        nc.tensor.matmul(ps[:], wt[:, k * P:(k + 1) * P],
                                         xh[k][:],
                                         start=(k == 0), stop=(k == KD - 1))
                    nc.scalar.activation(
                        dst[:, o * TOK:(o + 1) * TOK], ps[:], AF.Identity,
                        bias=vec_t[:, bias_col + o:bias_col + o + 1])

            for l in range(L):
                vec_t = sb.tile([P, NVEC], F32, tag="vec", bufs=2,
                                name=f"vec_{l}")
                nc.sync.dma_start(vec_t[:], vecs[l])

                # ---- K projection -> bounce (single merged K+V gather) ----
                KVW = KD * TOK + VSB  # 8256
                kv_src = dr.tile([P, KVW], BF16, name=f"kv_src_{l}")
                kv_all = dr.tile([GROUP, P, KVW], BF16, name=f"kv_all_{l}")
                k_sb = sb.tile([P, KD * TOK], BF16, tag="ksb", bufs=1,
                               name=f"ksb_{l}")
                proj(l, wk, C_BK, k_sb, "kt")
                with tc.high_priority():
                    nc.sync.dma_start(kv_src[:, 0:KD * TOK], k_sb[:])

                # ---- V projection (interleaved layout + ones) ----
                # v_sb[:, p*520 + t*130 + h*65 + f]; f==64 is the ones col.
                v_sb = sb.tile([P, VSB], BF16, tag="vsb", bufs=1,
                               name=f"vsb_{l}")
                v_view = v_sb[:].rearrange(
                    "p (pr t h f) -> p pr t h f", pr=NPAIR, t=4, h=2)
                # bake ones columns (per (hh, ph): dims [t, h])
                for pr in range(NPAIR):
                    nc.sync.dma_start(
                        v_view[:, pr, :, :, 64:65],
                        ones16d[:, 0:8].rearrange(
                            "p (t h o) -> p t h o", t=4, h=2))
                wvts = []
                for k in range(KD):
                    wvk = sb.tile([P, D], BF16, tag="wv8", bufs=8,
                                  name=f"wv_{l}_{k}")
                    nc.sync.dma_start(wvk[:], wv[l, k])
                    wvts.append(wvk)
                for t in range(4):
                    vps = psp.tile([P, D], F32, tag="psLG", bufs=2,
                                   name=f"ps_v_{l}_{t}")
                    for n in range(4):
                        for k in range(KD):
                            nc.tensor.matmul(
                                vps[:, n * 256:(n + 1) * 256],
                                xh[k][:, t * P:(t + 1) * P],
                                wvts[k][:, n * 256:(n + 1) * 256],
                                start=(k == 0), stop=(k == KD - 1))
                    for hh in range(2):
                        # dst dims [ph(4), h(2), f(64)]
                        nc.scalar.copy(
                            v_view[:, hh * 4:(hh + 1) * 4, t, :, 0:64],
                            vps[:, hh * 512:(hh + 1) * 512].rearrange(
                                "p (ph h f) -> p ph h f", ph=4, h=2))
                with tc.high_priority():
                    nc.sync.dma_start(kv_src[:, KD * TOK:], v_sb[:])
                    nc.gpsimd.collective_compute(
                        "AllGather", ALU.bypass, replica_groups=rg,
                        ins=[kv_src.opt()], outs=[kv_all.opt()])

                # ---- Q projection ----
                q_sb = sb.tile([P, KD * TOK], BF16, tag="qsb", bufs=1,
                               name=f"qsb_{l}")
                proj(l, wq, C_BQ, q_sb, "qt")

                # ---- attention ----
                ctx128 = []
                gate = None
                for p in range(NPAIR):
                    qs = q_sb[:, p * TOK:(p + 1) * TOK]
                    ctx = psp.tile([65, 2 * TOK], F32, tag="psCTX", bufs=1,
                                   name=f"ctx_{l}_{p}")
                    for c in range(GROUP):
                        ktile = sb.tile([P, TOK], BF16, tag="ktile", bufs=3,
                                        name=f"ktile_{l}_{p}_{c}")
                        nc.sync.dma_start(
                            ktile[:],
                            kv_all[c][:, p * TOK:(p + 1) * TOK])
                        vpc = sb.tile([P, VW], BF16, tag="vpc", bufs=3,
                                      name=f"vpc_{l}_{p}_{c}")
                        nc.sync.dma_start(
                            vpc[:],
                            kv_all[c][:, KD * TOK + p * VW:
                                        KD * TOK + (p + 1) * VW])
                        vv = vpc[:].rearrange("p (t h f) -> p t h f",
                                              t=4, h=2)
                        for j in range(4):
                            kt = c * 4 + j
                            lg = psp.tile([P, 2 * TOK], F32, tag="psLG",
                                          bufs=2, name=f"lg_{l}_{p}_{kt}")
                            nc.tensor.matmul(lg[:, 0:TOK],
                                             ktile[0:64, j * P:(j + 1) * P],
                                             qs[0:64, :])
                            nc.tensor.matmul(lg[:, TOK:2 * TOK],
                                             ktile[64:128, j * P:(j + 1) * P],
                                             qs[64:128, :])
                            probs = sb.tile([P, 2 * TOK], BF16, tag="probs",
                                            bufs=8,
                                            name=f"probs_{l}_{p}_{kt}")
                            nc.scalar.activation(probs[:], lg[:], AF.Exp,
                                                 scale=0.125,
                                                 bias=mb_t[:, kt:kt + 1])
                            va = vv[:, j, 0, :]
                            vb = vv[:, j, 1, :]
                            mm_a = nc.tensor.matmul(ctx[:, 0:TOK], va,
                                                    probs[:, 0:TOK],
                                                    start=(kt == 0),
                                                    stop=(kt == NKT - 1))
                            if gate is None:
                                gate = mm_a
                            nc.tensor.matmul(ctx[:, TOK:2 * TOK], vb,
                                             probs[:, TOK:2 * TOK],
                                             start=(kt == 0),
                                             stop=(kt == NKT - 1))

                    # normalize: sums live on psum row 64.  Copy the row
                    # out, reshape to [32, 32] via DMA, reciprocal on DVE,
                    # reshape back, then broadcast via K=1 matmuls.
                    srow = sb.tile([65, 2 * TOK], F32, tag="srow", bufs=1,
                                   name=f"srow_{l}_{p}")
                    nc.scalar.copy(srow[64:65, :], ctx[64:65, :])
                    rec32 = sb.tile([32, 32], F32, tag="rec32", bufs=2,
                                    name=f"rec32_{l}_{p}")
                    nc.sync.dma_start(r32(rec32[:]), r32(srow[64:65, :]))
                    rec32b = sb.tile([32, 32], F32, tag="rec32b", bufs=2,
                                     name=f"rec32b_{l}_{p}")
                    nc.vector.reciprocal(rec32b[:], rec32[:])
                    rsrow = sb.tile([1, 2 * TOK], F32, tag="rsrow", bufs=1,
                                    name=f"rsrow_{l}_{p}")
                    nc.sync.dma_start(r32(rsrow[:]), r32(rec32b[:]))
                    cx = sb.tile([P, TOK], BF16, tag="cx", bufs=8,
                                 name=f"cx_{l}_{p}")
                    ctx128.append(cx)
                    cxb = sb.tile([64, TOK], BF16, tag="cxb", bufs=2,
                                  name=f"cxb_{l}_{p}")
                    for hf in range(2):
                        rb = psp.tile([64, TOK], F32, tag="psA", bufs=2,
                                      name=f"rb_{l}_{p}_{hf}")
                        nc.tensor.matmul(
                            rb[:], r32(ones_row[:, 0:64]),
                            r32(rsrow[:, hf * TOK:(hf + 1) * TOK]))
                        rb_sb = sb.tile([64, TOK], F32, tag="rbsb", bufs=2,
                                        name=f"rbsb_{l}_{p}_{hf}")
                        nc.vector.tensor_copy(rb_sb[:], rb[:])
                        dst = cx[0:64, :] if hf == 0 else cxb[:]
                        nc.vector.tensor_mul(
                            dst, ctx[0:64, hf * TOK:(hf + 1) * TOK],
                            rb_sb[:])
                    nc.sync.dma_start(cx[64:128, :], cxb[:])

                # ---- Wo + residual -> r1 ----
                r1 = []
                for o in range(KD):
                    wt = sb.tile([P, D], BF16, tag="wproj", bufs=8,
                                 name=f"wo_{l}_{o}")
                    wdma = nc.sync.dma_start(wt[:], wo[l, o])
                    # keep the 19MB Wo/FFN weight prefetch out of the
                    # layer-start window where the K/V gather is in flight
                    tile.add_dep_helper(
                        wdma.ins if hasattr(wdma, "ins") else wdma,
                        gate.ins if hasattr(gate, "ins") else gate,
                        reason="delay weight prefetch past AG")
                    ps = psp.tile([P, TOK], F32, tag="psA", bufs=2,
                                  name=f"ps_wo_{l}_{o}")
                    for k in range(KD):
                        nc.tensor.matmul(ps[:], wt[:, k * P:(k + 1) * P],
                                         ctx128[k][:],
                                         start=(k == 0), stop=(k == KD - 1))
                    ao = sb.tile([P, TOK], F32, tag="aosb", bufs=2,
                                 name=f"ao_{l}_{o}")
                    nc.scalar.activation(
                        ao[:], ps[:], AF.Identity,
                        bias=vec_t[:, C_BATT + o:C_BATT + o + 1])
                    rt = sb.tile([P, TOK], F32, tag="res", bufs=16,
                                 name=f"r1_{l}_{o}")
                    nc.vector.tensor_add(r32(rt[:]), ao[:], xt[o][:])
                    r1.append(rt)

                def layernorm(src, gcol, bcol, phase):
                    """src: 8 fp32 tiles -> (xt_new fp32, xh_new bf16)."""
                    stm = psp.tile([1, TOK], F32, tag="psA", bufs=2,
                                   name=f"stm_{phase}")
                    sts = psp.tile([1, TOK], F32, tag="psA", bufs=2,
                                   name=f"sts_{phase}")
                    for k in range(KD):
                        nc.tensor.matmul(stm[:], r32(ones_col[:]),
                                         r32(src[k][:]),
                                         start=(k == 0), stop=(k == KD - 1))
                    for k in range(KD):
                        sq = sb.tile([P, TOK], F32, tag="sq", bufs=2,
                                     name=f"sq_{phase}_{k}")
                        nc.vector.tensor_mul(r32(sq[:]), src[k][:], src[k][:])
                        nc.tensor.matmul(sts[:], r32(ones_col[:]),
                                         r32(sq[:]),
                                         start=(k == 0), stop=(k == KD - 1))
                    stA = sb.tile([1, 3 * TOK], F32, tag="st", bufs=1,
                                  name=f"stA_{phase}")
                    stB = sb.tile([1, 2 * TOK], F32, tag="stB", bufs=1,
                                  name=f"stB_{phase}")
                    mean = stA[:, 0:TOK]
                    wk1 = stA[:, TOK:2 * TOK]
                    t3 = stA[:, 2 * TOK:3 * TOK]
                    rs = stB[:, 0:TOK]
                    murs = stB[:, TOK:2 * TOK]
                    nc.vector.tensor_scalar_mul(mean, stm[:], 1.0 / D)
                    nc.vector.tensor_scalar_mul(wk1, sts[:], 1.0 / D)
                    nc.vector.tensor_mul(t3, mean, mean)
                    nc.vector.tensor_sub(wk1, wk1, t3)
                    nc.vector.tensor_scalar_add(wk1, wk1, EPS)
                    nc.scalar.activation(wk1, wk1, AF.Ln)
                    nc.scalar.activation(r32(rs), wk1, AF.Exp, scale=-0.5)
                    nc.vector.tensor_mul(r32(murs), mean, rs)
                    bc = psp.tile([P, 2 * TOK], F32, tag="psLG", bufs=2,
                                  name=f"bc_{phase}")
                    nc.tensor.matmul(bc[:, 0:TOK], r32(ones_row[:]), r32(rs))
                    nc.tensor.matmul(bc[:, TOK:2 * TOK], r32(ones_row[:]),
                                     r32(murs))
                    xt_new, xh_new = [], []
                    for k in range(KD):
                        tb = sb.tile([P, TOK], F32, tag="lntmp", bufs=3,
                                     name=f"lnt_{phase}_{k}")
                        nc.vector.tensor_mul(tb[:], src[k][:], bc[:, 0:TOK])
                        nc.vector.tensor_sub(tb[:], tb[:],
                                             bc[:, TOK:2 * TOK])
                        nt = sb.tile([P, TOK], F32, tag="res", bufs=16,
                                     name=f"xt_{phase}_{k}")
                        nc.vector.tensor_scalar(
                            r32(nt[:]), tb[:], vec_t[:, gcol + k:gcol + k + 1],
                            vec_t[:, bcol + k:bcol + k + 1], ALU.mult, ALU.add)
                        nh = sb.tile([P, TOK], BF16, tag=f"xh{k}", bufs=1,
                                     name=f"xh_{phase}_{k}")
                        nc.vector.tensor_copy(nh[:], nt[:])
                        xt_new.append(nt)
                        xh_new.append(nh)
                    return xt_new, xh_new

                xt, xh = layernorm(r1, C_LG1, C_LB1, f"ln1_{l}")

                # ---- FFN ----
                nc.scalar.activation(dumt[:], dumt[:], AF.Gelu)
                h_sb = []
                for f in range(KF):
                    wt = sb.tile([P, D], BF16, tag="wproj", bufs=8,
                                 name=f"w1_{l}_{f}")
                    nc.sync.dma_start(wt[:], w1[l, f])
                    ps = psp.tile([P, TOK], F32, tag="psA", bufs=2,
                                  name=f"ps_f1_{l}_{f}")
                    for k in range(KD):
                        nc.tensor.matmul(ps[:], wt[:, k * P:(k + 1) * P],
                                         xh[k][:],
                                         start=(k == 0), stop=(k == KD - 1))
                    ht = sb.tile([P, TOK], BF16, tag="hbuf", bufs=32,
                                 name=f"h_{l}_{f}")
                    nc.scalar.activation(
                        ht[:], ps[:], AF.Gelu,
                        bias=vec_t[:, C_B1 + f:C_B1 + f + 1])
                    h_sb.append(ht)
                nc.scalar.activation(dumt[:], dumt[:], AF.Ln)
                r2 = []
                for o in range(KD):
                    ps = psp.tile([P, TOK], F32, tag="psA", bufs=2,
                                  name=f"ps_f2_{l}_{o}")
                    for qt in range(4):
                        wt = sb.tile([P, 1024], BF16, tag="wproj", bufs=8,
                                     name=f"w2_{l}_{o}_{qt}")
                        nc.sync.dma_start(wt[:], w2[l, o, qt])
                        for k in range(8):
                            kk = qt * 8 + k
                            nc.tensor.matmul(ps[:],
                                             wt[:, k * P:(k + 1) * P],
                                             h_sb[kk][:],
                                             start=(kk == 0), stop=(kk == 31))
                    fo = sb.tile([P, TOK], F32, tag="aosb", bufs=2,
                                 name=f"fo_{l}_{o}")
                    nc.scalar.activation(
                        fo[:], ps[:], AF.Identity,
                        bias=vec_t[:, C_B2 + o:C_B2 + o + 1])
                    rt = sb.tile([P, TOK], F32, tag="res", bufs=16,
                                 name=f"r2_{l}_{o}")
                    nc.vector.tensor_add(r32(rt[:]), fo[:], xt[o][:])
                    r2.append(rt)

                xt, xh = layernorm(r2, C_LG2, C_LB2, f"ln2_{l}")

            # ---- final layernorm (write fp32 output) ----
            vecf_t = sb.tile([P, 16], F32, tag="vec", bufs=2)
            nc.sync.dma_start(vecf_t[:], vecf[:])
            src = xt
            stm = psp.tile([1, TOK], F32, tag="psA", bufs=2, name="stm_f")
            sts = psp.tile([1, TOK], F32, tag="psA", bufs=2, name="sts_f")
            for k in range(KD):
                nc.tensor.matmul(stm[:], r32(ones_col[:]), r32(src[k][:]),
                                 start=(k == 0), stop=(k == KD - 1))
            for k in range(KD):
                sq = sb.tile([P, TOK], F32, tag="sq", bufs=2,
                             name=f"sq_f_{k}")
                nc.vector.tensor_mul(r32(sq[:]), src[k][:], src[k][:])
                nc.tensor.matmul(sts[:], r32(ones_col[:]), r32(sq[:]),
                                 start=(k == 0), stop=(k == KD - 1))
            stA = sb.tile([1, 3 * TOK], F32, tag="st", bufs=1, name="stA_f")
            stB = sb.tile([1, 2 * TOK], F32, tag="stB", bufs=1, name="stB_f")
            mean = stA[:, 0:TOK]
            wk1 = stA[:, TOK:2 * TOK]
            t3 = stA[:, 2 * TOK:3 * TOK]
            rs = stB[:, 0:TOK]
            murs = stB[:, TOK:2 * TOK]
            nc.vector.tensor_scalar_mul(mean, stm[:], 1.0 / D)
            nc.vector.tensor_scalar_mul(wk1, sts[:], 1.0 / D)
            nc.vector.tensor_mul(t3, mean, mean)
            nc.vector.tensor_sub(wk1, wk1, t3)
            nc.vector.tensor_scalar_add(wk1, wk1, EPS)
            nc.scalar.activation(wk1, wk1, AF.Ln)
            nc.scalar.activation(r32(rs), wk1, AF.Exp, scale=-0.5)
            nc.vector.tensor_mul(r32(murs), mean, rs)
            bc = psp.tile([P, 2 * TOK], F32, tag="psLG", bufs=2, name="bc_f")
            nc.tensor.matmul(bc[:, 0:TOK], r32(ones_row[:]), r32(rs))
            nc.tensor.matmul(bc[:, TOK:2 * TOK], r32(ones_row[:]), r32(murs))
            for k in range(KD):
                tb = sb.tile([P, TOK], F32, tag="lntmp", bufs=3,
                             name=f"lnt_f_{k}")
                nc.vector.tensor_mul(tb[:], src[k][:], bc[:, 0:TOK])
                nc.vector.tensor_sub(tb[:], tb[:], bc[:, TOK:2 * TOK])
                ot = sb.tile([P, TOK], F32, tag="aosb", bufs=2,
                             name=f"out_{k}")
                nc.vector.tensor_scalar(
                    ot[:], tb[:], vecf_t[:, k:k + 1],
                    vecf_t[:, 8 + k:8 + k + 1], ALU.mult, ALU.add)
                nc.sync.dma_start(outT[k], ot[:])

    nc.compile()
    return nc


def _get_program():
    global _PROGRAM
    if _PROGRAM is None:
        _PROGRAM = _build_program()
    return _PROGRAM


def _prep_host(inputs):
    """Build bf16 weight tiles + fp32 bias/affine vecs."""
    f = lambda a: np.asarray(a, dtype=np.float32)
    Wq, bq = f(inputs["Wq"]), f(inputs["bq"])
    Wk, bk = f(inputs["Wk"]), f(inputs["bk"])
    Wv, bv = f(inputs["Wv"]), f(inputs["bv"])
    Wo, bo = f(inputs["Wo"]), f(inputs["bo"])
    f1w, f1b = f(inputs["fc1_w"]), f(inputs["fc1_b"])
    f2w, f2b = f(inputs["fc2_w"]), f(inputs["fc2_b"])
    ln1_g, ln1_b = f(inputs["ln1_g"]), f(inputs["ln1_b"])
    ln2_g, ln2_b = f(inputs["ln2_g"]), f(inputs["ln2_b"])
    lnf_g, lnf_b = f(inputs["lnf_g"]), f(inputs["lnf_b"])

    def proj_tiles(WT):  # WT [D_in, M_out] -> [M_out/P][P, D_in] lhsT blocks
        kd = WT.shape[0] // P
        n_o = WT.shape[1] // P
        a = WT.reshape(kd, P, n_o, P).transpose(2, 1, 0, 3)
        return np.ascontiguousarray(a.reshape(n_o, P, kd * P))

    wq_h = np.empty((L, KD, P, D), BF16NP)
    wk_h = np.empty((L, KD, P, D), BF16NP)
    wv_h = np.empty((L, KD, P, D), BF16NP)
    wo_h = np.empty((L, KD, P, D), BF16NP)
    w1_h = np.empty((L, KF, P, D), BF16NP)
    w2_h = np.empty((L, KD, 4, P, 1024), BF16NP)
    vecs_h = np.empty((L, P, NVEC), np.float32)

    def cols(v):  # [dim] -> [P, dim/P] per-tile columns
        return np.ascontiguousarray(v.reshape(-1, P).T.astype(np.float32))

    for l in range(L):
        b_att = bo[l] + Wo[l] @ bv[l]
        wq_h[l] = proj_tiles(Wq[l].T)
        wk_h[l] = proj_tiles(Wk[l].T)
        wv_h[l] = Wv[l].T.reshape(KD, P, D)
        wo_h[l] = proj_tiles(Wo[l].T)
        w1_h[l] = proj_tiles(f1w[l].T)
        t2 = proj_tiles(f2w[l].T)  # [KD][P, DFF]
        w2_h[l] = t2.reshape(KD, P, 4, 1024).transpose(0, 2, 1, 3).astype(
            BF16NP)
        v = vecs_h[l]
        v[:, C_BQ:C_BQ + 8] = cols(bq[l])
        v[:, C_BK:C_BK + 8] = cols(bk[l])
        v[:, C_BATT:C_BATT + 8] = cols(b_att)
        v[:, C_B1:C_B1 + 32] = cols(f1b[l])
        v[:, C_B2:C_B2 + 8] = cols(f2b[l])
        v[:, C_LG1:C_LG1 + 8] = cols(ln1_g[l])
        v[:, C_LB1:C_LB1 + 8] = cols(ln1_b[l])
        v[:, C_LG2:C_LG2 + 8] = cols(ln2_g[l])
        v[:, C_LB2:C_LB2 + 8] = cols(ln2_b[l])

    vecf_h = np.empty((P, 16), np.float32)
    vecf_h[:, 0:8] = cols(lnf_g)
    vecf_h[:, 8:16] = cols(lnf_b)

    return dict(wq=wq_h, wk=wk_h, wv=wv_h, wo=wo_h, w1=w1_h, w2=w2_h,
                vecs=vecs_h, vecf=vecf_h)


def kernel(**inputs):
    global LAST_RES
    nc = _get_program()
    shared = _prep_host(inputs)
    x = np.asarray(inputs["x"], dtype=np.float32)
    mask = np.asarray(inputs["mask"])

    in_maps = []
    for c in range(NCORES):
        b, s = c // GROUP, c % GROUP
        xTc = np.ascontiguousarray(
            x[b, s * TOK:(s + 1) * TOK, :].T).reshape(KD, P, TOK)
        mbv = (mask[b].astype(np.float32) - 1.0) * 30.0
        mb_c = np.ascontiguousarray(mbv.reshape(NKT, P).T.astype(np.float32))
        m = dict(shared)
        m["xT"] = xTc
        m["x16"] = xTc.astype(BF16NP)
        m["mb"] = mb_c
        m["onesd"] = np.ones((P, 8), np.float32)
        m["ones16d"] = np.ones((P, 8), BF16NP)
        in_maps.append(m)

    res = run_bass_kernel_spmd(nc, in_maps, list(range(NCORES)))
    LAST_RES = res
    out = np.empty((B, T, D), np.float32)
    for c in range(NCORES):
        b, s = c // GROUP, c % GROUP
        oT = res.results[c]["outT"].reshape(D, TOK)
        out[b, s * TOK:(s + 1) * TOK, :] = oT.T
    return out
